# revision 7
# baseline (speedup 1.0000x reference)
"""Causal self-attention (B=2,T=2048,C=1024,H=16) on 8 trn2 cores — v2.

Sharding: core = (batch b, head-group g); b = core//4, g = core%4.
Each core computes attention for 4 heads of one batch plus the
row-parallel slice of c_proj; host sums the 4 partial projections per
batch and adds b_proj (+ the c_proj image of the v-bias, which is a
constant vector and exact to fold host-side).

v2 changes vs v1:
- bf16 matmul operands throughout (measured 173 vs 317 ns/MM at N=512).
- k-side qkv bias dropped (constant per query -> cancels in softmax).
- v-bias folded into the host-side constant (sum_k P = 1 after norm).
- score matmuls for the two heads of a partition pair issued to PE row
  groups (0,0)/(64,0) back-to-back so they run concurrently.
- one exp per (pair, k-tile) over [128, 2, 512] (bigger Act instrs).
- per-pair batched normalization (denom rows -> recip -> broadcast MM).
- c_proj for window j delayed until after attention window j+1 starts.
- bf16 output, converted + reduced on host.
"""

import os
import numpy as np
import ml_dtypes
from contextlib import ExitStack

import concourse.bass as bass
import concourse.mybir as mybir
import concourse.tile as tile
from concourse.bass import ts, ds
from concourse.bass_utils import run_bass_kernel_spmd
from concourse.vector_clock import ScopedClock

# ---------------------------------------------------------------------------
# Workaround: walrus CoreV3 rejects >2 sem waits on one instruction; the
# TileContext exit drain accumulates one wait per outstanding proc.  Split
# them across single-wait sync nops before the drain.
# ---------------------------------------------------------------------------


def _patched_drain_and_barrier(self, tick_clock, wait_clock):
    nc = self.nc
    probe = mybir.InstNoOp(name=nc.get_next_instruction_name(), ins=[], outs=[])
    probe.engine = mybir.EngineType.SP
    wait_clock.add_sem_waits(probe, ScopedClock({None: tick_clock.global_clock}))
    waits = list(probe.sync_info.on_wait) if probe.sync_info else []
    for w in waits:
        n = nc.sync.nop(nofuse=True, hint="drain_wait_split")
        n.ins.sync_info = mybir.SyncInfo(on_wait=[w], on_update=[])
    nc.sync.drain()
    nc.all_engine_barrier()
    assert self.sems is not None
    popped = nc._tile_sem_poison_stack.pop()
    assert popped is self._sem_poison
    nc.clear_and_free_semaphores(list(self.sems.allocated().values()))
    nc.all_engine_barrier()


tile.TileContext._drain_and_barrier = _patched_drain_and_barrier

_DMA_INSTS = (
    mybir.InstCollectiveCompute,
)


def split_excess_waits(nc):
    """walrus CoreV3 encodes at most 1 sem wait per compute instruction
    (2 on EventSemaphore); hoist extras onto same-engine nops."""
    for fn in nc.m.functions:
        for bb in fn.blocks:
            insts = bb.instructions
            new_list = []
            changed = False
            for inst in insts:
                si = inst.sync_info
                cap = 2 if isinstance(inst, mybir.InstEventSemaphore) else 1
                if (
                    si is not None
                    and not isinstance(inst, _DMA_INSTS)
                    and len(si.on_wait) > cap
                ):
                    waits = list(si.on_wait)
                    extra, keep = waits[:-cap], waits[-cap:]
                    for w in extra:
                        nop = mybir.InstNoOp(
                            name=nc.get_next_instruction_name(), ins=[], outs=[]
                        )
                        nop.engine = inst.engine
                        nop.sync_info = mybir.SyncInfo(on_wait=[w], on_update=[])
                        nc.register_instruction(nop)
                        new_list.append(nop)
                    inst.sync_info = mybir.SyncInfo(
                        on_wait=keep, on_update=list(si.on_update)
                    )
                    changed = True
                new_list.append(inst)
            if changed:
                bb.instructions = new_list

# ---------------------------------------------------------------------------

B, T, C, H, HD = 2, 2048, 1024, 16, 64
NCORES, GROUPS = 8, 4
CL = C // GROUPS          # 256 channels (4 heads) per core
HPC = H // GROUPS         # 4 heads per core

QT = 512                  # q window
NQW = T // QT             # 4 q windows
NKT = T // 128            # 16 k tiles of 128

F32 = mybir.dt.float32
BF16 = mybir.dt.bfloat16
AF = mybir.ActivationFunctionType
OP = mybir.AluOpType


def build_nc():
    reps = int(os.environ.get("KREPS", "1"))
    nc = bass.Bass()
    xT = nc.dram_tensor("xT", [C, T], BF16, kind="ExternalInput")
    wqkT = nc.dram_tensor("wqkT", [C, 2 * CL], BF16, kind="ExternalInput")
    wvT = nc.dram_tensor("wvT", [C, CL], BF16, kind="ExternalInput")
    wpT = nc.dram_tensor("wpT", [CL, C], BF16, kind="ExternalInput")
    bq2 = nc.dram_tensor("bq2", [128, 2], F32, kind="ExternalInput")
    m0 = nc.dram_tensor("m0", [128, 128], BF16, kind="ExternalInput")
    ones64 = nc.dram_tensor("ones64", [128, 64], BF16, kind="ExternalInput")
    outp = nc.dram_tensor("outp", [T, C], BF16, kind="ExternalOutput")

    with tile.TileContext(nc) as tc, ExitStack() as ctx, nc.allow_low_precision(
        reason="bf16 attention; graded at rel tol 2e-2"
    ):
        persist = ctx.enter_context(tc.tile_pool(name="persist", bufs=1))
        work = ctx.enter_context(tc.tile_pool(name="work", bufs=3))
        nrm = ctx.enter_context(tc.tile_pool(name="nrm", bufs=2))
        outsb = ctx.enter_context(tc.tile_pool(name="outsb", bufs=2))
        psS = ctx.enter_context(tc.tile_pool(name="psS", bufs=2, space="PSUM"))
        psY = ctx.enter_context(tc.tile_pool(name="psY", bufs=3, space="PSUM"))
        psD = ctx.enter_context(tc.tile_pool(name="psD", bufs=1, space="PSUM"))

        # persistent tensors
        xTs = persist.tile([128, 8, T], BF16)
        qkT = persist.tile([128, 4, T], BF16)        # o-tiles: q01 q23 k01 k23
        vaug = persist.tile([128, NKT, HPC, HD + 1], BF16)
        yT = persist.tile([128, 2, T], BF16)
        wqk_s = persist.tile([128, 8, 2 * CL], BF16)
        wv_s = persist.tile([128, 8, CL], BF16)
        wp_s = persist.tile([128, 2, C], BF16)
        bq_s = persist.tile([128, 2], F32)
        m0_s = persist.tile([128, 128], BF16)
        ones_r = persist.tile([1, 64], BF16)

        nc.sync.dma_start(out=wqk_s, in_=wqkT.rearrange("(cc p) o -> p cc o", p=128))
        nc.sync.dma_start(out=wv_s, in_=wvT.rearrange("(cc p) o -> p cc o", p=128))
        nc.sync.dma_start(out=wp_s, in_=wpT.rearrange("(cc p) o -> p cc o", p=128))
        nc.gpsimd.dma_start(out=bq_s, in_=bq2[:, :])
        nc.gpsimd.dma_start(out=m0_s, in_=m0[:, :])
        nc.gpsimd.dma_start(out=ones_r, in_=ones64[0:1, 0:64])
        nc.gpsimd.dma_start(
            out=vaug[:, :, :, HD:HD + 1],
            in_=ones64.rearrange("p (a b) -> p a b", b=HPC).unsqueeze(3),
        )
        # x by halves so compute can start after the first 4MB
        xT_r = xT.rearrange("(cc p) t -> p cc t", p=128)
        for th in range(2):
            nc.sync.dma_start(
                out=xTs[:, :, ts(th, T // 2)], in_=xT_r[:, :, ts(th, T // 2)]
            )

        def qk_phase(th):
            # q^T/k^T for tokens [th*1024, (th+1)*1024); W stationary
            # reused across the two 512-token slices.
            for o in range(4):
                ps = psS.tile([128, 2, QT], F32, tag="s")
                for cc in range(8):
                    for ti in range(2):
                        t = th * 2 + ti
                        nc.tensor.matmul(
                            ps[:, ti, :],
                            lhsT=wqk_s[:, cc, ts(o, 128)],
                            rhs=xTs[:, cc, ts(t, QT)],
                            start=(cc == 0),
                            stop=(cc == 7),
                        )
                if o < 2:
                    nc.vector.tensor_scalar(
                        out=qkT[:, o, ds(th * 1024, 1024)].rearrange(
                            "p (a b) -> p a b", a=2
                        ),
                        in0=ps,
                        scalar1=1.0,
                        scalar2=bq_s[:, o:o + 1],
                        op0=OP.mult,
                        op1=OP.add,
                    )
                else:
                    nc.vector.tensor_copy(
                        out=qkT[:, o, ds(th * 1024, 1024)].rearrange(
                            "p (a b) -> p a b", a=2
                        ),
                        in_=ps,
                    )

        def v_phase(th):
            # v for k-tiles [8*th, 8*th+8); x-tile stationary.
            for ttp in range(4):
                psv = psY.tile([128, 2, CL], F32, tag="y")
                for i in range(2):
                    tt = th * 8 + ttp * 2 + i
                    for cc in range(8):
                        nc.tensor.matmul(
                            psv[:, i, :],
                            lhsT=xTs[:, cc, ts(tt, 128)],
                            rhs=wv_s[:, cc, :],
                            start=(cc == 0),
                            stop=(cc == 7),
                        )
                for i in range(2):
                    nc.vector.tensor_copy(
                        out=vaug[:, th * 8 + ttp * 2 + i, :, 0:HD],
                        in_=psv[:, i, :].rearrange("p (h d) -> p h d", h=HPC),
                    )

        def attn_window(j):
            nkt = 4 * (j + 1)
            for hp in range(2):
                psy = [
                    psY.tile([65, QT], F32, name=f"psy{hp}{w}", tag="y")
                    for w in range(2)
                ]
                pending = []
                for kt in range(nkt):
                    m = kt - 4 * j
                    q_lo = m * 128 if m >= 0 else 0
                    n = QT - q_lo
                    pss = psS.tile([128, 2, QT], F32, tag="s")
                    pt = work.tile([128, 2, QT], BF16, tag="pt")
                    for w in range(2):
                        nc.tensor.matmul(
                            pss[:, w, q_lo:QT],
                            lhsT=qkT[ds(64 * w, 64), 2 + hp, ts(kt, 128)],
                            rhs=qkT[ds(64 * w, 64), hp, ds(j * QT + q_lo, n)],
                            start=True,
                            stop=True,
                        )
                    nc.scalar.activation(
                        out=pt[:, :, q_lo:QT], in_=pss[:, :, q_lo:QT], func=AF.Exp
                    )
                    if m >= 0:
                        for w in range(2):
                            nc.vector.tensor_mul(
                                out=pt[:, w, ds(q_lo, 128)],
                                in0=pt[:, w, ds(q_lo, 128)],
                                in1=m0_s,
                            )
                    pending.append((pt, kt, q_lo))
                    if len(pending) > 2:
                        pv(*pending.pop(0), hp=hp, nkt=nkt, psy=psy)
                for args in pending:
                    pv(*args, hp=hp, nkt=nkt, psy=psy)
                # ---- batched pair normalization ----
                dn = nrm.tile([128, QT], F32, tag="dn")
                ps_dn = psD.tile([128, QT], F32, tag="dn")
                for w in range(2):
                    rc = nrm.tile([1, QT], BF16, name=f"rc{w}", tag=f"rc{w}")
                    nc.vector.reciprocal(out=rc, in_=psy[w][64:65, :])
                    nc.tensor.matmul(
                        ps_dn[ds(64 * w, 64), :],
                        lhsT=ones_r,
                        rhs=rc,
                        start=True,
                        stop=True,
                    )
                nc.scalar.copy(out=dn, in_=ps_dn)
                for w in range(2):
                    nc.vector.tensor_mul(
                        out=yT[ds(64 * w, 64), hp, ts(j, QT)],
                        in0=psy[w][0:64, :],
                        in1=dn[ds(64 * w, 64), :],
                    )

        def pv(pt, kt, q_lo, hp, nkt, psy):
            for w in range(2):
                nc.tensor.matmul(
                    psy[w][:, q_lo:QT],
                    lhsT=vaug[:, kt, 2 * hp + w, :],
                    rhs=pt[:, w, q_lo:QT],
                    start=(kt == 0),
                    stop=(kt == nkt - 1),
                )

        def cproj_window(j):
            for tl in range(4):
                tt = j * 4 + tl
                ob = outsb.tile([128, C], BF16, tag="ob")
                pso = [
                    psY.tile([128, QT], F32, name=f"pso{nn_}", tag="y")
                    for nn_ in range(2)
                ]
                for c2 in range(2):
                    for nn_ in range(2):
                        nc.tensor.matmul(
                            pso[nn_],
                            lhsT=yT[:, c2, ts(tt, 128)],
                            rhs=wp_s[:, c2, ts(nn_, QT)],
                            start=(c2 == 0),
                            stop=(c2 == 1),
                        )
                for nn_ in range(2):
                    nc.vector.tensor_copy(out=ob[:, ts(nn_, QT)], in_=pso[nn_])
                nc.sync.dma_start(out=outp[ts(tt, 128), :], in_=ob)

        sections = os.environ.get("KSECTIONS", "full")
        for rep in range(reps):
            qk_phase(0)
            v_phase(0)
            if sections != "qkv":
                attn_window(0)
            qk_phase(1)
            v_phase(1)
            if sections != "qkv":
                attn_window(1)
                if sections == "full":
                    cproj_window(0)
                attn_window(2)
                if sections == "full":
                    cproj_window(1)
                attn_window(3)
                if sections == "full":
                    cproj_window(2)
                    cproj_window(3)

    split_excess_waits(nc)
    return nc


_NC_CACHE = None


def _get_nc():
    global _NC_CACHE
    if _NC_CACHE is None:
        _NC_CACHE = build_nc()
    return _NC_CACHE


def make_in_maps(x, W_attn, b_attn, W_proj):
    x = np.asarray(x, np.float32)
    W_attn = np.asarray(W_attn, np.float32)
    b_attn = np.asarray(b_attn, np.float32)
    W_proj = np.asarray(W_proj, np.float32)
    bf = ml_dtypes.bfloat16
    m0 = np.triu(np.ones((128, 128), np.float32)).astype(bf)  # keep q >= k
    in_maps = []
    for core in range(NCORES):
        b, g = core // GROUPS, core % GROUPS
        qr = slice(g * CL, (g + 1) * CL)
        kr = slice(C + g * CL, C + (g + 1) * CL)
        vr = slice(2 * C + g * CL, 2 * C + (g + 1) * CL)
        # q-side prescaled by 1/8; k-side bias dropped (cancels in softmax)
        wqk = np.concatenate([W_attn[qr] / 8.0, W_attn[kr]], axis=0)  # [512, 1024]
        in_maps.append({
            "xT": np.ascontiguousarray(x[b].T).astype(bf),
            "wqkT": np.ascontiguousarray(wqk.T).astype(bf),
            "wvT": np.ascontiguousarray(W_attn[vr].T).astype(bf),
            "wpT": np.ascontiguousarray(W_proj[:, g * CL:(g + 1) * CL].T).astype(bf),
            "bq2": np.ascontiguousarray((b_attn[qr] / 8.0).reshape(2, 128).T).astype(
                np.float32
            ),
            "m0": m0,
            "ones64": np.ones((128, 64), np.float32).astype(bf),
        })
    return in_maps


def kernel(x, W_attn, b_attn, W_proj, b_proj, **_unused):
    nc = _get_nc()
    in_maps = make_in_maps(x, W_attn, b_attn, W_proj)
    res = run_bass_kernel_spmd(nc, in_maps, core_ids=list(range(NCORES)))
    out = np.zeros((B, T, C), np.float32)
    for core in range(NCORES):
        out[core // GROUPS] += np.asarray(res.results[core]["outp"], np.float32)
    bv = np.asarray(b_attn, np.float32)[2 * C:3 * C]
    const = np.asarray(b_proj, np.float32) + np.asarray(W_proj, np.float32) @ bv
    out += const[None, None, :]
    return out
